# revision 32
# baseline (speedup 1.0000x reference)
"""Trainium2 Bass kernel for nn_AUAttnProcessor (AU-token attention processor).

Sharding: 8 cores = (batch b, head-group hg). Core c handles batch c//2 and
heads [4*(c%2), 4*(c%2)+4) (Ch=320 of C=640 channels).  Wq/Wk/Wv/Wak/Wav are
column-sharded, Wo row-sharded; each core emits a partial [S, C] output and the
host reduces the two partials per batch and adds bias + residual.

v2 design (all-bf16, flash-style transposed attention, batched normalization):
  qT/kT = (Wslice.T @ hsT)            [80, S] per head  (C contraction on partitions)
  vaug  = hs @ Wv_slice               [128, sc, h, 97] bf16, ones col at 96
  scoresT[kc] = kT_chunk x qT         PSUM [128, QB]
  expT = Exp(scoresT) -> bf16         (no max subtraction; scores are O(1))
  outT += vaug_chunk x expT           PSUM [97, QB]; row 96 = softmax denom
  mainT/auout stored unnormalized; all 8 denominator rows batched through one
  reciprocal_approx_fast per q-block, PE-broadcast to 80 partitions, merged on
  DVE, then merged x Wo_rows -> partial [S, C].  Host adds partials + bias +
  residual.  Merge/Wo work of q-block 0 is interleaved into q-block 1's
  attention to keep the PE dense (HAM stays warm).
"""

import os
import sys

import numpy as np

for _p in ("/opt/trn_rl_repo",):
    if os.path.isdir(_p) and _p not in sys.path:
        sys.path.insert(0, _p)

import concourse.bass as bass
import concourse.tile as tile
from concourse import bacc, mybir
from concourse.bass_utils import run_bass_kernel_spmd

# Problem dims
B, S, C, H, D = 4, 2048, 640, 8, 80
NH = 4            # heads per core
CH = NH * D       # 320 channels per core
KC = C // 128     # 5 contraction chunks
SC = S // 128     # 16 sequence chunks
NAU = 13          # 12 AU tokens + 1 null token
NAUP = 14         # padded to even size (pad row is zero)
QB = 1024         # q-block width for main attention
NQB = S // QB
SCALE = float(D) ** -0.5

F32 = mybir.dt.float32
BF16 = mybir.dt.bfloat16
EXP = mybir.ActivationFunctionType.Exp


def build_nc(iters=1):
    nc = bacc.Bacc()
    hsT = nc.dram_tensor("hsT", [C, S], F32, kind="ExternalInput")
    wq = nc.dram_tensor("wq", [C, CH], F32, kind="ExternalInput")
    wk = nc.dram_tensor("wk", [C, CH], F32, kind="ExternalInput")
    wv = nc.dram_tensor("wv", [C, CH], F32, kind="ExternalInput")
    wak = nc.dram_tensor("wak", [C, CH], F32, kind="ExternalInput")
    wav = nc.dram_tensor("wav", [C, CH], F32, kind="ExternalInput")
    wo = nc.dram_tensor("wo", [CH, C], F32, kind="ExternalInput")
    extT = nc.dram_tensor("extT", [C, NAUP], F32, kind="ExternalInput")
    extzT = nc.dram_tensor("extzT", [C, NAUP], F32, kind="ExternalInput")
    sel8c = nc.dram_tensor("sel8c", [8, 8 * D], F32, kind="ExternalInput")
    outp = nc.dram_tensor("outp", [S, C], F32, kind="ExternalOutput")
    ld = nc.gpsimd  # casting f32->bf16 loads need SWDGE

    import contextlib
    with tile.TileContext(nc) as tc, \
         nc.allow_low_precision(reason="bf16 attention; approx reciprocal"), \
         (tc.For_i(0, iters, 1) if iters > 1 else contextlib.nullcontext()):
        with tc.tile_pool(name="pers", bufs=1) as pers:
            qT = pers.tile([D, NH, S], BF16, name="qT")
            kT = pers.tile([D, NH, S], BF16, name="kT")
            # v with ones col at 96 (softmax denominator); cols 80:96 zero pad
            vaug = pers.tile([128, SC, NH, 97], BF16, name="vaug")
            wo_sb = pers.tile([D, NH, C], BF16, name="wo_sb")
            aukT = pers.tile([D, NH, NAUP], BF16, name="aukT")
            auvaug = pers.tile([NAUP, NH, 98], BF16, name="auvaug")
            # sel8[:, r, :] is an [8, D] selection matrix with row r all-ones:
            # sel8[:, r, :].T @ recs[8, N] broadcasts recs row r to D partitions
            sel8 = pers.tile([8, 8, D], BF16, name="sel8")
            # e8[0:1, r, :] is a [1, 8] one-hot row-r vector: accumulating
            # e8[0:1,r,:].T @ srows[0:1,r,:] over r stacks the single-partition
            # sum rows into an [8, N] PSUM tile (engine writes to partitions
            # 1..7 are illegal, so the stacking must go through the PE)
            e8 = pers.tile([1, 8, 8], BF16, name="e8")
            srows = pers.tile([1, 8, S], BF16, name="srows")  # 0:4 main, 4:8 au
            mainT = pers.tile([D, NH, S], BF16, name="mainT")
            auout = pers.tile([D, NH, S], BF16, name="auout")


            nc.vector.memset(vaug[:, :, :, 80:96], 0.0)
            nc.vector.memset(vaug[:, :, :, 96:97], 1.0)
            nc.vector.memset(e8, 0.0)
            for r in range(8):
                nc.vector.memset(e8[0:1, r, r:r + 1], 1.0)
            # auvaug pad layout: [80:96]=0, [96]=ones (rows 0:13 only -- the
            # padded 14th key must not enter the softmax denominator), [97]=0
            nc.vector.memset(auvaug[:, :, 80:98], 0.0)
            nc.vector.memset(auvaug[0:NAU, :, 96:97], 1.0)

            # ---------------- Phase A: projections ----------------
            with tc.tile_pool(name="projp", bufs=1) as projp, \
                 tc.tile_pool(name="wts", bufs=5) as wpool, \
                 tc.tile_pool(name="ppsum", bufs=4, space="PSUM") as pps:
                wq_sb = wpool.tile([128, KC, CH], BF16, tag="w", name="wq_sb")
                ld.dma_start(out=wq_sb, in_=wq[:].rearrange("(c p) n -> p c n", p=128))
                wk_sb = wpool.tile([128, KC, CH], BF16, tag="w", name="wk_sb")
                ld.dma_start(out=wk_sb, in_=wk[:].rearrange("(c p) n -> p c n", p=128))
                # hsT loaded in four S-chunks so q/k matmuls start early
                hsT_sb = projp.tile([128, KC, S], BF16, name="hsT_sb")
                for nb in range(4):
                    sl = slice(nb * 512, (nb + 1) * 512)
                    ld.dma_start(
                        out=hsT_sb[:, :, sl],
                        in_=hsT[:, sl].rearrange("(c p) s -> p c s", p=128),
                    )
                wv_sb = wpool.tile([128, KC, CH], BF16, tag="w", name="wv_sb")
                ld.dma_start(out=wv_sb, in_=wv[:].rearrange("(c p) n -> p c n", p=128))
                ld.dma_start(out=wo_sb, in_=wo[:].rearrange("(h d) n -> d h n", d=D))
                ext_sb = projp.tile([128, KC, NAUP], BF16, name="ext_sb")
                ld.dma_start(out=ext_sb, in_=extT[:].rearrange("(c p) n -> p c n", p=128))
                extz_sb = projp.tile([128, KC, NAUP], BF16, name="extz_sb")
                ld.dma_start(out=extz_sb, in_=extzT[:].rearrange("(c p) n -> p c n", p=128))
                ld.dma_start(out=sel8, in_=sel8c[:].rearrange("p (r d) -> p r d", d=D))
                wak_sb = wpool.tile([128, KC, CH], BF16, tag="w", name="wak_sb")
                ld.dma_start(out=wak_sb, in_=wak[:].rearrange("(c p) n -> p c n", p=128))
                wav_sb = wpool.tile([128, KC, CH], BF16, tag="w", name="wav_sb")
                ld.dma_start(out=wav_sb, in_=wav[:].rearrange("(c p) n -> p c n", p=128))

                COPY = mybir.ActivationFunctionType.Copy

                # q and k projections per hsT chunk (transposed output);
                # evacuation on the otherwise-idle ScalarE
                for nb in range(S // 512):
                    for w_sb, dstT in ((wq_sb, qT), (wk_sb, kT)):
                        for h in range(NH):
                            ps = pps.tile([D, 512], F32, tag="pp", name="ps_qk")
                            for c in range(KC):
                                nc.tensor.matmul(
                                    ps,
                                    w_sb[:, c, h * D:(h + 1) * D],
                                    hsT_sb[:, c, nb * 512:(nb + 1) * 512],
                                    start=(c == 0), stop=(c == KC - 1),
                                )
                            nc.scalar.activation(
                                out=dstT[:, h, nb * 512:(nb + 1) * 512], in_=ps,
                                func=COPY,
                            )

                # au_k projection (transposed, per head)
                for h in range(NH):
                    ps = pps.tile([D, NAUP], F32, tag="pp", name="ps_auk")
                    for c in range(KC):
                        nc.tensor.matmul(
                            ps,
                            wak_sb[:, c, h * D:(h + 1) * D],
                            ext_sb[:, c, :],
                            start=(c == 0), stop=(c == KC - 1),
                        )
                    nc.vector.tensor_copy(aukT[:, h, :], ps)

                # au_v projection (natural [14, 320], gamma pre-folded on host)
                ps = pps.tile([NAUP, CH], F32, tag="pp", name="ps_auv")
                for c in range(KC):
                    nc.tensor.matmul(
                        ps,
                        extz_sb[:, c, :],
                        wav_sb[:, c, :],
                        start=(c == 0), stop=(c == KC - 1),
                    )
                nc.vector.tensor_copy(
                    auvaug[:, :, 0:80], ps.rearrange("p (h d) -> p h d", d=D)
                )

                # v projection (natural layout, strided into vaug) interleaved
                # with the AU-token attention (fills ACT/VEC while PE runs)
                with tc.tile_pool(name="aups", bufs=1, space="PSUM") as aups, \
                     tc.tile_pool(name="auop", bufs=1, space="PSUM") as auop, \
                     tc.tile_pool(name="auep", bufs=2) as auep:

                    def au_h(h):
                        """AU-token cross attention for one head (2 q halves)."""
                        for half in range(2):
                            hs_ = slice(half * QB, (half + 1) * QB)
                            aus = aups.tile([NAUP, QB], F32, tag="aus", name="aus")
                            for nn in range(QB // 512):
                                q0 = half * QB + nn * 512
                                nc.tensor.matmul(
                                    aus[:, nn * 512:(nn + 1) * 512],
                                    aukT[:, h, :],
                                    qT[:, h, q0:q0 + 512],
                                    start=True, stop=True,
                                )
                            au_e = auep.tile([NAUP, QB], BF16, tag="aue", name="au_e")
                            nc.scalar.activation(out=au_e, in_=aus, func=EXP)
                            auo = auop.tile([98, QB], F32, tag="auo", name="auo")
                            for nn in range(QB // 512):
                                nc.tensor.matmul(
                                    auo[:, nn * 512:(nn + 1) * 512],
                                    auvaug[:, h, :],
                                    au_e[:, nn * 512:(nn + 1) * 512],
                                    start=True, stop=True,
                                )
                            nc.vector.tensor_copy(auout[:, h, hs_], auo[0:80, :])
                            nc.vector.tensor_copy(srows[0:1, 4 + h, hs_], auo[96:97, :])

                    for sc in range(SC):
                        ps = pps.tile([128, CH], F32, tag="pp", name="ps_v")
                        for c in range(KC):
                            nc.tensor.matmul(
                                ps,
                                hsT_sb[:, c, sc * 128:(sc + 1) * 128],
                                wv_sb[:, c, :],
                                start=(c == 0), stop=(c == KC - 1),
                            )
                        nc.vector.tensor_copy(
                            vaug[:, sc, :, 0:80], ps.rearrange("p (h d) -> p h d", d=D)
                        )
                        if sc % 4 == 3:
                            au_h(sc // 4)

            # ------- Phase C/E: main attention + merge + Wo, one pool scope -------
            with tc.tile_pool(name="spool", bufs=2, space="PSUM") as spool, \
                 tc.tile_pool(name="opool", bufs=2, space="PSUM") as opool, \
                 tc.tile_pool(name="expp", bufs=3) as expp, \
                 tc.tile_pool(name="mpool", bufs=2) as mpool, \
                 tc.tile_pool(name="scrp", bufs=2) as scrp, \
                 tc.tile_pool(name="recp", bufs=3) as recp, \
                 tc.tile_pool(name="outp_sb", bufs=3) as outsb_pool:

                def attn_qh(qb, h):
                    """scores -> exp -> PV for one (q-block, head); unnormalized."""
                    q0 = qb * QB
                    outT = opool.tile([97, QB], F32, tag="ot", name="outT")
                    for kc in range(SC):
                        sco = spool.tile([128, QB], F32, tag="sc", name="sco")
                        for nn in range(QB // 512):
                            nc.tensor.matmul(
                                sco[:, nn * 512:(nn + 1) * 512],
                                kT[:, h, kc * 128:(kc + 1) * 128],
                                qT[:, h, q0 + nn * 512:q0 + (nn + 1) * 512],
                                start=True, stop=True,
                            )
                        ex = expp.tile([128, QB], BF16, tag="ex", name="ex")
                        nc.scalar.activation(out=ex, in_=sco, func=EXP)
                        for nn in range(QB // 512):
                            nc.tensor.matmul(
                                outT[:, nn * 512:(nn + 1) * 512],
                                vaug[:, kc, h, :],
                                ex[:, nn * 512:(nn + 1) * 512],
                                start=(kc == 0), stop=(kc == SC - 1),
                            )
                    nc.vector.tensor_copy(mainT[:, h, q0:q0 + QB], outT[0:80, :])
                    nc.vector.tensor_copy(srows[0:1, h, q0:q0 + QB], outT[96:97, :])

                def stack_recip_qh(qb, h):
                    """stack the (main, au) denom rows for (qb, h) -> 1/x -> bf16."""
                    q0 = qb * QB
                    s2p = spool.tile([2, QB], F32, tag="sc", name="s2p")
                    for r, row in ((0, h), (1, 4 + h)):
                        for nn in range(QB // 512):
                            nc.tensor.matmul(
                                s2p[:, nn * 512:(nn + 1) * 512],
                                e8[0:1, r, 0:2],
                                srows[0:1, row, q0 + nn * 512:q0 + (nn + 1) * 512],
                                start=(r == 0), stop=(r == 1),
                            )
                    rec2f = recp.tile([2, QB], F32, tag="rf", name="rec2f")
                    nc.vector.reciprocal_approx_fast(out=rec2f, in_=s2p)
                    rec2b = recp.tile([2, QB], BF16, tag="rb", name="rec2b")
                    nc.vector.tensor_copy(rec2b, rec2f)
                    return rec2b

                def merge_qh(qb, h, merged, rec2b):
                    """broadcast 1/denom to 80 partitions, merge main+au."""
                    q0 = qb * QB
                    parts = []
                    for r, src in ((0, mainT), (1, auout)):
                        bc = spool.tile([D, QB], F32, tag="sc", name="bc")
                        for nn in range(QB // 512):
                            nc.tensor.matmul(
                                bc[:, nn * 512:(nn + 1) * 512],
                                sel8[0:2, r, :],
                                rec2b[:, nn * 512:(nn + 1) * 512],
                                start=True, stop=True,
                            )
                        t = scrp.tile([D, QB], BF16, tag="t%d" % r, name="t")
                        nc.vector.tensor_mul(t, src[:, h, q0:q0 + QB], bc)
                        parts.append(t)
                    nc.vector.tensor_add(merged[:, h, :], parts[0], parts[1])

                def wo_qb(qb, merged):
                    q0 = qb * QB
                    for sj in range(QB // 128):
                        wo_ps = opool.tile([128, 2, 512], F32, tag="ot", name="wo_ps")
                        for nn in range(2):
                            for h in range(NH):
                                nc.tensor.matmul(
                                    wo_ps[:, nn, 0:320],
                                    merged[:, h, sj * 128:(sj + 1) * 128],
                                    wo_sb[:, h, nn * 320:(nn + 1) * 320],
                                    start=(h == 0), stop=(h == NH - 1),
                                )
                        o_sb = outsb_pool.tile([128, 2, 320], F32, tag="ob", name="o_sb")
                        nc.vector.tensor_copy(o_sb, wo_ps[:, :, 0:320])
                        s0 = q0 + sj * 128
                        nc.sync.dma_start(out=outp[s0:s0 + 128, :], in_=o_sb)

                # Pipeline: AU(h) rides inside qb0's head loop; each head's
                # denominators are stacked+inverted immediately; merges lag one
                # head so their broadcast matmuls never head-block the PE FIFO.
                merged0 = mpool.tile([D, NH, QB], BF16, tag="mg", name="merged0")
                merged1 = mpool.tile([D, NH, QB], BF16, tag="mg", name="merged1")
                recs = {}
                for h in range(NH):
                    attn_qh(0, h)
                    recs[(0, h)] = stack_recip_qh(0, h)
                    if h > 0:
                        merge_qh(0, h - 1, merged0, recs.pop((0, h - 1)))
                for h in range(NH):
                    attn_qh(1, h)
                    recs[(1, h)] = stack_recip_qh(1, h)
                    if h == 0:
                        merge_qh(0, NH - 1, merged0, recs.pop((0, NH - 1)))
                        wo_qb(0, merged0)
                    else:
                        merge_qh(1, h - 1, merged1, recs.pop((1, h - 1)))
                merge_qh(1, NH - 1, merged1, recs.pop((1, NH - 1)))
                wo_qb(1, merged1)
    nc.compile()
    return nc


_NC_CACHE = {}
LAST_EXEC_NS = None


def _get_nc():
    if "nc" not in _NC_CACHE:
        _NC_CACHE["nc"] = build_nc()
    return _NC_CACHE["nc"]


def make_in_maps(inputs):
    hs = np.asarray(inputs["hidden_states"], np.float32)
    au = np.asarray(inputs["au_embedding"], np.float32)
    Wq = np.asarray(inputs["Wq"], np.float32)
    Wk = np.asarray(inputs["Wk"], np.float32)
    Wv = np.asarray(inputs["Wv"], np.float32)
    Wak = np.asarray(inputs["Wak"], np.float32)
    Wav = np.asarray(inputs["Wav"], np.float32)
    null_token = np.asarray(inputs["null_token"], np.float32).reshape(1, C)
    gamma = np.asarray(inputs["gamma"], np.float32)
    Wo = np.asarray(inputs["Wo"], np.float32)

    Wq_s = Wq * SCALE
    Wav_g = Wav * gamma[None, :]
    sel = np.zeros((8, 8, D), np.float32)
    for r in range(8):
        sel[r, r, :] = 1.0
    sel = np.ascontiguousarray(sel.reshape(8, 8 * D))

    in_maps = []
    for c in range(8):
        b, hg = divmod(c, 2)
        sl = slice(hg * CH, (hg + 1) * CH)
        ext = np.concatenate(
            [au[b], null_token, np.zeros((1, C), np.float32)], axis=0
        )  # [14, C]; row 13 is even-size padding
        extz = ext.copy()
        extz[NAU - 1] = 0.0
        in_maps.append({
            "hsT": np.ascontiguousarray(hs[b].T),
            "wq": np.ascontiguousarray(Wq_s[:, sl]),
            "wk": np.ascontiguousarray(Wk[:, sl]),
            "wv": np.ascontiguousarray(Wv[:, sl]),
            "wak": np.ascontiguousarray(Wak[:, sl]),
            "wav": np.ascontiguousarray(Wav_g[:, sl]),
            "wo": np.ascontiguousarray(Wo[sl, :]),
            "extT": np.ascontiguousarray(ext.T),
            "extzT": np.ascontiguousarray(extz.T),
            "sel8c": sel,
        })
    return in_maps


def kernel(**inputs):
    global LAST_EXEC_NS
    hs = np.asarray(inputs["hidden_states"], np.float32)
    bo = np.asarray(inputs["bo"], np.float32)
    in_maps = make_in_maps(inputs)
    nc = _get_nc()
    trace = os.environ.get("KERNEL_TRACE", "0") == "1"
    res = run_bass_kernel_spmd(nc, in_maps, list(range(8)), trace=trace)
    LAST_EXEC_NS = res.exec_time_ns
    out = np.empty((B, S, C), np.float32)
    for b in range(B):
        out[b] = res.results[2 * b]["outp"] + res.results[2 * b + 1]["outp"]
        out[b] += bo[None, :]
        out[b] += hs[b]
    return out
